# revision 8
# baseline (speedup 1.0000x reference)
"""Causal MHA (B=4, T=2048, D=1024, H=16) on 8 trn2 cores.

Sharding: core c = (batch b = c//2, head-group g = c%2). Each core computes
QKV projections for its 8 heads, causal attention, and the row-parallel
out-proj partial product. Host sums the two partials per batch + bias.

Schedule: q-chunk j (512 queries) OUTER, head-pair cp INNER ("j-outer").
Per j: S^T tiles stream through PE->ScalarE(exp); projections for chunk
j+1 and out-proj for chunk j-1 drain as queued PE work between S steps,
so the tensor engine never starves and HAM stays at full clock.
Diagonal S tiles are narrowed to the causal range (less PE/ACT/mask work).

On-device layout (per core):
  xT   [1024, 2048]  X^T (d on partitions)           bf16
  QT_j [512, 512]    Q^T for current q-chunk         bf16 (2 bufs)
  KT   [512, 2048]   K^T (e=head*64+d rows)          bf16
  V_pad [2048, 520]  V natural + ones col per head   bf16
  scores S^T tiles [128 k, 2x512 q] (2 heads/psum), exp on ScalarE,
  ctx = P^T-stationary matmul -> [128 q, 65] (col 64 = softmax denom),
  normalize per-partition, DMA-transpose -> ctx^T, out-proj partial.
"""

import os

import numpy as np
import ml_dtypes

import concourse.bass as bass
import concourse.bacc as bacc
import concourse.tile as tile
from concourse import mybir
from concourse.bass_utils import run_bass_kernel_spmd
from concourse.masks import make_identity

BF16 = ml_dtypes.bfloat16

B, T, D = 4, 2048, 1024
H, HD = 16, 64
E = 512          # per-core projection width (8 heads * 64)
DC = D // 128    # 8 contraction chunks
EC = E // 128    # 4 e chunks (head pairs)
TJ = T // 512    # 4 q-chunks of 512
TQ = T // 128    # 16 t-chunks of 128

F32 = mybir.dt.float32
BF = mybir.dt.bfloat16

LAST = {}
_CACHE = {}


def _build():
    nc = bacc.Bacc("TRN2")
    xT = nc.dram_tensor("xT", [D, T], BF, kind="ExternalInput")
    wq = nc.dram_tensor("wq", [D, E], BF, kind="ExternalInput")
    wk = nc.dram_tensor("wk", [D, E], BF, kind="ExternalInput")
    wv = nc.dram_tensor("wv", [D, E], BF, kind="ExternalInput")
    wo = nc.dram_tensor("wo", [E, D], BF, kind="ExternalInput")
    outp = nc.dram_tensor("out", [T, D], F32, kind="ExternalOutput")

    with tile.TileContext(nc) as tc:
        with (
            tc.tile_pool(name="const", bufs=1) as const,
            tc.tile_pool(name="acts", bufs=1) as acts,
            tc.tile_pool(name="ppool", bufs=26) as ppool,
            tc.tile_pool(name="small", bufs=6) as small,
            tc.tile_pool(name="stage", bufs=6) as stage,
            tc.tile_pool(name="obuf", bufs=2) as obufp,
            tc.tile_pool(name="psS", bufs=2, space="PSUM") as psS,
            tc.tile_pool(name="psP", bufs=2, space="PSUM") as psP,
            tc.tile_pool(name="psC", bufs=2, space="PSUM") as psC,
        ):
            # ---------- constants; DMA order = first-need order:
            # wq/wk + xT chunk0 gate the j=0 streams, wv next (ctx j=0),
            # remaining xT chunks gate proj of chunk j+1, wo only at outproj
            wq_sb = const.tile([128, DC, E], BF, tag="wq")
            wk_sb = const.tile([128, DC, E], BF, tag="wk")
            wv_sb = const.tile([128, DC, E], BF, tag="wv")
            wo_sb = const.tile([128, EC, D], BF, tag="wo")
            ident = const.tile([128, 128], BF, tag="ident")
            xT_sb = acts.tile([128, DC, T], BF, tag="xT")

            # weights ride the ScalarE DMA queue (idle at startup); xT rides
            # Sync in contiguous [128,1024] chunks, first-needed first
            nc.scalar.dma_start(
                out=wq_sb, in_=wq.rearrange("(dc p) e -> p dc e", p=128)
            )
            nc.scalar.dma_start(
                out=wk_sb, in_=wk.rearrange("(dc p) e -> p dc e", p=128)
            )
            for th in range(2):
                for dc in range(DC):
                    nc.sync.dma_start(
                        out=xT_sb[:, dc, th * 1024 : (th + 1) * 1024],
                        in_=xT[dc * 128 : (dc + 1) * 128, th * 1024 : (th + 1) * 1024],
                    )
            nc.scalar.dma_start(
                out=wv_sb, in_=wv.rearrange("(dc p) e -> p dc e", p=128)
            )
            nc.scalar.dma_start(
                out=wo_sb, in_=wo.rearrange("(ec p) o -> p ec o", p=128)
            )
            make_identity(nc, ident)

            # trigger the exp table load immediately (overlaps input DMA)
            actwarm = small.tile([128, 1], F32, tag="actwarm")
            nc.scalar.activation(
                out=actwarm,
                in_=ident[:, 0:1],
                func=mybir.ActivationFunctionType.Exp,
            )

            # spin the PE on the identity during the input DMA so HAM
            # un-throttles before real work arrives (~5us of N=128 matmuls)
            psW = psP.tile([128, 512], F32, tag="psP")
            for _ in range(48):
                nc.tensor.matmul(
                    psW[:, 0:128], lhsT=ident, rhs=ident, start=True, stop=True
                )

            KT_sb = acts.tile([128, EC, T], BF, tag="KT")
            V_sb = acts.tile([128, TQ, 8 * 65], BF, tag="V")
            CT_sb = acts.tile([128, EC, T], BF, tag="CT")

            # ones columns (col 64 of each per-head 65-group): one strided memset
            nc.vector.memset(
                V_sb.rearrange("p t (h d) -> p t h d", d=65)[:, :, :, 64:65], 1.0
            )

            qt_tiles = {}

            def get_qt(j):
                if j not in qt_tiles:
                    qt_tiles[j] = acts.tile(
                        [128, EC, 512], BF, tag="QT", bufs=2, name=f"QT{j}"
                    )
                return qt_tiles[j]

            # ---------- op factories ----------
            def q_proj_op(cp, j):
                def op(cp=cp, j=j):
                    qt = get_qt(j)
                    ps = psP.tile([128, 512], F32, tag="psP")
                    for dc in range(DC):
                        nc.tensor.matmul(
                            ps,
                            lhsT=wq_sb[:, dc, cp * 128 : (cp + 1) * 128],
                            rhs=xT_sb[:, dc, j * 512 : (j + 1) * 512],
                            start=(dc == 0),
                            stop=(dc == DC - 1),
                        )
                    nc.vector.tensor_copy(out=qt[:, cp, :], in_=ps)

                return op

            def k_proj_op(cp, j):
                def op(cp=cp, j=j):
                    ps = psP.tile([128, 512], F32, tag="psP")
                    for dc in range(DC):
                        nc.tensor.matmul(
                            ps,
                            lhsT=wk_sb[:, dc, cp * 128 : (cp + 1) * 128],
                            rhs=xT_sb[:, dc, j * 512 : (j + 1) * 512],
                            start=(dc == 0),
                            stop=(dc == DC - 1),
                        )
                    nc.vector.tensor_copy(
                        out=KT_sb[:, cp, j * 512 : (j + 1) * 512], in_=ps
                    )

                return op

            def v_proj_op(t7):
                def op(t7=t7):
                    ps = psP.tile([128, 512], F32, tag="psP")
                    for dc in range(DC):
                        nc.tensor.matmul(
                            ps,
                            lhsT=xT_sb[:, dc, t7 * 128 : (t7 + 1) * 128],
                            rhs=wv_sb[:, dc, :],
                            start=(dc == 0),
                            stop=(dc == DC - 1),
                        )
                    nc.vector.tensor_copy(
                        out=V_sb.rearrange("p t (h d) -> p t h d", d=65)[
                            :, t7, :, 0:64
                        ],
                        in_=ps.rearrange("p (h d) -> p h d", d=64),
                    )

                return op

            def qk_proj_ops(j, cps=None):
                ops = []
                for cp in cps if cps is not None else range(EC):
                    ops.append(q_proj_op(cp, j))
                    ops.append(k_proj_op(cp, j))
                return ops

            def v_proj_ops(j):
                return [v_proj_op(t7) for t7 in range(4 * j, 4 * j + 4)]

            def proj_ops(j):
                return qk_proj_ops(j) + v_proj_ops(j)

            def outproj_ops(j):
                ops = []
                for t7 in range(4 * j, 4 * j + 4):

                    def op(t7=t7):
                        ob = obufp.tile([128, 1024], F32, tag="obuf")
                        for oc in range(2):
                            ps = psP.tile([128, 512], F32, tag="psP")
                            for ec in range(EC):
                                nc.tensor.matmul(
                                    ps,
                                    lhsT=CT_sb[:, ec, t7 * 128 : (t7 + 1) * 128],
                                    rhs=wo_sb[:, ec, oc * 512 : (oc + 1) * 512],
                                    start=(ec == 0),
                                    stop=(ec == EC - 1),
                                )
                            nc.vector.tensor_copy(
                                out=ob[:, oc * 512 : (oc + 1) * 512], in_=ps
                            )
                        nc.sync.dma_start(
                            out=outp[t7 * 128 : (t7 + 1) * 128, :], in_=ob
                        )

                    ops.append(op)
                return ops

            def ctx_ops(cp, j, pts):
                ops = []
                for qr in range(4):
                    qc = 4 * j + qr
                    cn = stage.tile([128, 128], BF, tag="ctxn")
                    for h in range(2):

                        def mm_group(cp=cp, h=h, qr=qr, qc=qc, j=j, pts=pts, cn=cn):
                            habs = 2 * cp + h
                            cps = psC.tile([128, 65], F32, tag="psC")
                            for i in range(qc + 1):
                                nc.tensor.matmul(
                                    cps,
                                    lhsT=pts[i][:, h, qr * 128 : (qr + 1) * 128],
                                    rhs=V_sb[:, i, habs * 65 : habs * 65 + 65],
                                    start=(i == 0),
                                    stop=(i == qc),
                                )
                            rc = small.tile([128, 1], F32, tag="recip")
                            nc.vector.reciprocal(rc, cps[:, 64:65])
                            nc.vector.tensor_scalar_mul(
                                out=cn[:, 64 * h : 64 * h + 64],
                                in0=cps[:, 0:64],
                                scalar1=rc,
                            )

                        ops.append(mm_group)

                    def finish(cp=cp, qc=qc, cn=cn):
                        nc.sync.dma_start_transpose(
                            out=CT_sb[:, cp, qc * 128 : (qc + 1) * 128], in_=cn
                        )

                    ops.append(finish)
                return ops

            # ---------- pipelined emission ----------
            # chunk-0 projections emitted inline (they gate the first stream)
            for op in proj_ops(0):
                op()

            slow = list(proj_ops(1))
            fast = []
            for j in range(TJ):
                nk = 4 * j + 4
                steps_left = 4 * nk
                for cp in range(EC):
                    if j == TJ - 1:
                        # chunk-3 projections are staggered into j=3: stream
                        # (cp,3) drains the projections needed by (cp+1,3)
                        if cp == 0:
                            slow += v_proj_ops(3) + qk_proj_ops(3, cps=[1])
                        elif cp < EC - 1:
                            slow += qk_proj_ops(3, cps=[cp + 1])
                        steps_left = nk
                    fper = (len(fast) + max(1, nk // 2) - 1) // max(1, nk // 2)
                    qt = get_qt(j)
                    pts = []
                    for i in range(nk):
                        r = i - 4 * j  # >=0 on the diagonal block
                        lo_q = 128 * r if r > 0 else 0
                        pt = ppool.tile([128, 2, 512], BF, tag="P")
                        sh = psS.tile([128, 2, 512], F32, tag="psS")
                        for h in range(2):
                            lo = 64 * h
                            nc.tensor.matmul(
                                sh[:, h, lo_q:512],
                                lhsT=KT_sb[lo : lo + 64, cp, i * 128 : (i + 1) * 128],
                                rhs=qt[lo : lo + 64, cp, lo_q:512],
                                start=True,
                                stop=True,
                            )
                        nc.scalar.activation(
                            out=pt[:, :, lo_q:512],
                            in_=sh[:, :, lo_q:512],
                            func=mybir.ActivationFunctionType.Exp,
                            scale=0.125,
                        )
                        if r >= 0:  # diagonal 128x128: keep f >= p, else 0
                            for h in range(2):
                                nc.gpsimd.affine_select(
                                    out=pt[:, h, 128 * r : 128 * r + 128],
                                    in_=pt[:, h, 128 * r : 128 * r + 128],
                                    compare_op=mybir.AluOpType.is_ge,
                                    fill=0.0,
                                    base=0,
                                    pattern=[[1, 128]],
                                    channel_multiplier=-1,
                                )
                        pts.append(pt)
                        for _ in range(fper):
                            if fast:
                                fast.pop(0)()
                        spr = (len(slow) + steps_left - 1) // steps_left
                        for _ in range(spr):
                            if slow:
                                slow.pop(0)()
                        steps_left -= 1
                    while fast:
                        fast.pop(0)()
                    fast = ctx_ops(cp, j, pts)
                # end of q-chunk j: stage next work into the queues
                if j == 0:
                    slow += qk_proj_ops(2)
                elif j == 1:
                    slow += v_proj_ops(2) + outproj_ops(0) + qk_proj_ops(3, cps=[0])
                elif j == 2:
                    slow += outproj_ops(1) + outproj_ops(2)
                if j == TJ - 1:
                    # tail: interleave the last ctx with its out-proj rows
                    op4 = outproj_ops(3)
                    mix = []
                    for qr in range(4):
                        mix.extend(fast[qr * 3 : qr * 3 + 3])
                        mix.append(op4[qr])
                    fast = mix
            while fast:
                fast.pop(0)()
            while slow:
                slow.pop(0)()
    nc.compile()
    return nc


def _get_nc():
    if "nc" not in _CACHE:
        _CACHE["nc"] = _build()
    return _CACHE["nc"]


def _ensure_ntff_hook():
    """Install the axon NTFF profiling hook if the image's antenv lacks it."""
    import sys
    import types

    try:
        import antenv.axon_hooks  # noqa: F401

        return
    except ImportError:
        pass
    try:
        import antenv

        mod = types.ModuleType("antenv.axon_hooks")
        holder = {"hook": None}
        mod.set_axon_ntff_profile_hook = lambda h: holder.__setitem__("hook", h)
        mod.get_axon_ntff_profile_hook = lambda: holder["hook"]
        sys.modules["antenv.axon_hooks"] = mod
        antenv.axon_hooks = mod
        from trn_agent_boot.trn_boot import _ntff_profile_via_ctypes

        so = "/opt/axon/libaxon_pjrt.so"
        if os.path.exists(so):
            mod.set_axon_ntff_profile_hook(_ntff_profile_via_ctypes(so))
    except Exception:
        pass


def kernel(inputs, Wq, Wk, Wv, Wo, bo):
    inputs = np.asarray(inputs, dtype=np.float32)
    Wq = np.asarray(Wq, dtype=np.float32)
    Wk = np.asarray(Wk, dtype=np.float32)
    Wv = np.asarray(Wv, dtype=np.float32)
    Wo = np.asarray(Wo, dtype=np.float32)
    bo = np.asarray(bo, dtype=np.float32)

    nc = _get_nc()
    wqs = [np.ascontiguousarray(Wq[:, g * E : (g + 1) * E]).astype(BF16) for g in range(2)]
    wks = [np.ascontiguousarray(Wk[:, g * E : (g + 1) * E]).astype(BF16) for g in range(2)]
    wvs = [np.ascontiguousarray(Wv[:, g * E : (g + 1) * E]).astype(BF16) for g in range(2)]
    wos = [np.ascontiguousarray(Wo[g * E : (g + 1) * E, :]).astype(BF16) for g in range(2)]
    xTs = [np.ascontiguousarray(inputs[b].T).astype(BF16) for b in range(B)]

    in_maps = []
    for c in range(8):
        b, g = divmod(c, 2)
        in_maps.append(
            {
                "xT": xTs[b],
                "wq": wqs[g],
                "wk": wks[g],
                "wv": wvs[g],
                "wo": wos[g],
            }
        )

    trace = os.environ.get("KERNEL_TRACE", "0") == "1"
    if trace:
        _ensure_ntff_hook()
    tcores = None
    if os.environ.get("KERNEL_TRACE_ALL", "0") == "1":
        tcores = list(range(8))
    res = run_bass_kernel_spmd(
        nc, in_maps, core_ids=list(range(8)), trace=trace, trace_cores=tcores
    )
    LAST["exec_ns"] = res.exec_time_ns
    LAST["trace"] = res.instructions_and_trace
    LAST["profile_json"] = res.profile_json

    out = np.empty((B, T, D), np.float32)
    for b in range(B):
        out[b] = res.results[2 * b]["out"] + res.results[2 * b + 1]["out"] + bo[None, :]
    return out


# revision 12
# speedup vs baseline: 1.0615x; 1.0615x over previous
"""Causal MHA (B=4, T=2048, D=1024, H=16) on 8 trn2 cores.

Sharding: core c = (batch b = c//2, head-group g = c%2). Each core computes
QKV projections for its 8 heads, causal attention, and the row-parallel
out-proj partial product. Host sums the two partials per batch + bias.

Schedule: q-chunk j (512 queries) OUTER, head-pair cp INNER ("j-outer").
Per j: S^T tiles stream through PE->ScalarE(exp); projections for chunk
j+1 and out-proj for chunk j-1 drain as queued PE work between S steps,
so the tensor engine never starves and HAM stays at full clock.
Diagonal S tiles are narrowed to the causal range (less PE/ACT/mask work).

On-device layout (per core):
  xT   [1024, 2048]  X^T (d on partitions)           bf16
  QT_j [512, 512]    Q^T for current q-chunk         bf16 (2 bufs)
  KT   [512, 2048]   K^T (e=head*64+d rows)          bf16
  V_pad [2048, 520]  V natural + ones col per head   bf16
  scores S^T tiles [128 k, 2x512 q] (2 heads/psum), exp on ScalarE,
  ctx = P^T-stationary matmul -> [128 q, 65] (col 64 = softmax denom),
  normalize per-partition, DMA-transpose -> ctx^T, out-proj partial.
"""

import os

import numpy as np
import ml_dtypes

import concourse.bass as bass
import concourse.bacc as bacc
import concourse.tile as tile
from concourse import mybir
from concourse.bass_utils import run_bass_kernel_spmd
from concourse.masks import make_identity

BF16 = ml_dtypes.bfloat16

B, T, D = 4, 2048, 1024
H, HD = 16, 64
E = 512          # per-core projection width (8 heads * 64)
DC = D // 128    # 8 contraction chunks
EC = E // 128    # 4 e chunks (head pairs)
TJ = T // 512    # 4 q-chunks of 512
TQ = T // 128    # 16 t-chunks of 128

F32 = mybir.dt.float32
BF = mybir.dt.bfloat16

LAST = {}
_CACHE = {}


def _build():
    nc = bacc.Bacc("TRN2")
    xT = nc.dram_tensor("xT", [D, T], BF, kind="ExternalInput")
    wq = nc.dram_tensor("wq", [D, E], BF, kind="ExternalInput")
    wk = nc.dram_tensor("wk", [D, E], BF, kind="ExternalInput")
    wv = nc.dram_tensor("wv", [D, E], BF, kind="ExternalInput")
    wo = nc.dram_tensor("wo", [E, D], BF, kind="ExternalInput")
    outp = nc.dram_tensor("out", [T, D], F32, kind="ExternalOutput")

    with tile.TileContext(nc) as tc:
        with (
            tc.tile_pool(name="const", bufs=1) as const,
            tc.tile_pool(name="acts", bufs=1) as acts,
            tc.tile_pool(name="ppool", bufs=26) as ppool,
            tc.tile_pool(name="small", bufs=6) as small,
            tc.tile_pool(name="stage", bufs=6) as stage,
            tc.tile_pool(name="obuf", bufs=2) as obufp,
            tc.tile_pool(name="psS", bufs=2, space="PSUM") as psS,
            tc.tile_pool(name="psP", bufs=2, space="PSUM") as psP,
            tc.tile_pool(name="psC", bufs=2, space="PSUM") as psC,
        ):
            # ---------- constants; DMA order = first-need order:
            # wq/wk + xT chunk0 gate the j=0 streams, wv next (ctx j=0),
            # remaining xT chunks gate proj of chunk j+1, wo only at outproj
            wq_sb = const.tile([128, DC, E], BF, tag="wq")
            wk_sb = const.tile([128, DC, E], BF, tag="wk")
            wv_sb = const.tile([128, DC, E], BF, tag="wv")
            wo_sb = const.tile([128, EC, D], BF, tag="wo")
            ident = const.tile([128, 128], BF, tag="ident")
            xT_sb = acts.tile([128, DC, T], BF, tag="xT")

            # weights ride the ScalarE DMA queue (idle at startup); xT rides
            # Sync in contiguous [128,1024] chunks, first-needed first
            nc.scalar.dma_start(
                out=wq_sb, in_=wq.rearrange("(dc p) e -> p dc e", p=128)
            )
            nc.scalar.dma_start(
                out=wk_sb, in_=wk.rearrange("(dc p) e -> p dc e", p=128)
            )
            for th in range(2):
                for dc in range(DC):
                    nc.sync.dma_start(
                        out=xT_sb[:, dc, th * 1024 : (th + 1) * 1024],
                        in_=xT[dc * 128 : (dc + 1) * 128, th * 1024 : (th + 1) * 1024],
                    )
            nc.scalar.dma_start(
                out=wv_sb, in_=wv.rearrange("(dc p) e -> p dc e", p=128)
            )
            nc.scalar.dma_start(
                out=wo_sb, in_=wo.rearrange("(ec p) o -> p ec o", p=128)
            )
            make_identity(nc, ident)

            # trigger the exp table load immediately (overlaps input DMA)
            actwarm = small.tile([128, 1], F32, tag="actwarm")
            nc.scalar.activation(
                out=actwarm,
                in_=ident[:, 0:1],
                func=mybir.ActivationFunctionType.Exp,
            )

            # spin the PE on the identity during the input DMA so HAM
            # un-throttles before real work arrives (~5us of N=128 matmuls)
            psW = psP.tile([128, 512], F32, tag="psP")
            for _ in range(64):
                nc.tensor.matmul(
                    psW[:, 0:128], lhsT=ident, rhs=ident, start=True, stop=True
                )

            KT_sb = acts.tile([128, EC, T], BF, tag="KT")
            V_sb = acts.tile([128, TQ, 8 * 65], BF, tag="V")
            CT_sb = acts.tile([128, EC, T], BF, tag="CT")

            # ones columns (col 64 of each per-head 65-group): one strided memset
            nc.vector.memset(
                V_sb.rearrange("p t (h d) -> p t h d", d=65)[:, :, :, 64:65], 1.0
            )

            qt_tiles = {}

            def get_qt(j):
                if j not in qt_tiles:
                    qt_tiles[j] = acts.tile(
                        [128, EC, 512], BF, tag="QT", bufs=2, name=f"QT{j}"
                    )
                return qt_tiles[j]

            # ---------- op factories ----------
            def q_proj_op(cp, j):
                def op(cp=cp, j=j):
                    qt = get_qt(j)
                    ps = psP.tile([128, 512], F32, tag="psP")
                    for dc in range(DC):
                        nc.tensor.matmul(
                            ps,
                            lhsT=wq_sb[:, dc, cp * 128 : (cp + 1) * 128],
                            rhs=xT_sb[:, dc, j * 512 : (j + 1) * 512],
                            start=(dc == 0),
                            stop=(dc == DC - 1),
                        )
                    nc.vector.tensor_copy(out=qt[:, cp, :], in_=ps)

                return op

            def k_proj_op(cp, j):
                def op(cp=cp, j=j):
                    ps = psP.tile([128, 512], F32, tag="psP")
                    for dc in range(DC):
                        nc.tensor.matmul(
                            ps,
                            lhsT=wk_sb[:, dc, cp * 128 : (cp + 1) * 128],
                            rhs=xT_sb[:, dc, j * 512 : (j + 1) * 512],
                            start=(dc == 0),
                            stop=(dc == DC - 1),
                        )
                    nc.vector.tensor_copy(
                        out=KT_sb[:, cp, j * 512 : (j + 1) * 512], in_=ps
                    )

                return op

            def v_proj_op(t7):
                def op(t7=t7):
                    ps = psP.tile([128, 512], F32, tag="psP")
                    for dc in range(DC):
                        nc.tensor.matmul(
                            ps,
                            lhsT=xT_sb[:, dc, t7 * 128 : (t7 + 1) * 128],
                            rhs=wv_sb[:, dc, :],
                            start=(dc == 0),
                            stop=(dc == DC - 1),
                        )
                    nc.vector.tensor_copy(
                        out=V_sb.rearrange("p t (h d) -> p t h d", d=65)[
                            :, t7, :, 0:64
                        ],
                        in_=ps.rearrange("p (h d) -> p h d", d=64),
                    )

                return op

            def qk_proj_ops(j, cps=None):
                ops = []
                for cp in cps if cps is not None else range(EC):
                    ops.append(q_proj_op(cp, j))
                    ops.append(k_proj_op(cp, j))
                return ops

            def v_proj_ops(j):
                return [v_proj_op(t7) for t7 in range(4 * j, 4 * j + 4)]

            def proj_ops(j):
                return qk_proj_ops(j) + v_proj_ops(j)

            def outproj_ops(j, scalar_copy=False):
                ops = []
                for t7 in range(4 * j, 4 * j + 4):

                    def op(t7=t7, scalar_copy=scalar_copy):
                        ob = obufp.tile([128, 1024], F32, tag="obuf")
                        for oc in range(2):
                            ps = psP.tile([128, 512], F32, tag="psP")
                            for ec in range(EC):
                                nc.tensor.matmul(
                                    ps,
                                    lhsT=CT_sb[:, ec, t7 * 128 : (t7 + 1) * 128],
                                    rhs=wo_sb[:, ec, oc * 512 : (oc + 1) * 512],
                                    start=(ec == 0),
                                    stop=(ec == EC - 1),
                                )
                            if scalar_copy:
                                nc.scalar.copy(
                                    out=ob[:, oc * 512 : (oc + 1) * 512], in_=ps
                                )
                            else:
                                nc.vector.tensor_copy(
                                    out=ob[:, oc * 512 : (oc + 1) * 512], in_=ps
                                )
                        nc.sync.dma_start(
                            out=outp[t7 * 128 : (t7 + 1) * 128, :], in_=ob
                        )

                    ops.append(op)
                return ops

            def ctx_ops(cp, j, pts):
                ops = []
                for qr in range(4):
                    qc = 4 * j + qr
                    cn = stage.tile([128, 128], BF, tag="ctxn")
                    for h in range(2):

                        def mm_group(cp=cp, h=h, qr=qr, qc=qc, j=j, pts=pts, cn=cn):
                            habs = 2 * cp + h
                            cps = psC.tile([128, 65], F32, tag="psC")
                            for i in range(qc + 1):
                                nc.tensor.matmul(
                                    cps,
                                    lhsT=pts[i][:, h, qr * 128 : (qr + 1) * 128],
                                    rhs=V_sb[:, i, habs * 65 : habs * 65 + 65],
                                    start=(i == 0),
                                    stop=(i == qc),
                                )
                            rc = small.tile([128, 1], F32, tag="recip")
                            nc.vector.reciprocal(rc, cps[:, 64:65])
                            nc.vector.tensor_scalar_mul(
                                out=cn[:, 64 * h : 64 * h + 64],
                                in0=cps[:, 0:64],
                                scalar1=rc,
                            )

                        ops.append(mm_group)

                    def finish(cp=cp, qc=qc, cn=cn):
                        nc.sync.dma_start_transpose(
                            out=CT_sb[:, cp, qc * 128 : (qc + 1) * 128], in_=cn
                        )

                    ops.append(finish)
                return ops

            # ---------- pipelined emission ----------
            # only the first stream's projections go inline; everything else
            # drains between S steps so ScalarE starts exp'ing ASAP
            for op in qk_proj_ops(0, cps=[0]):
                op()

            slow = (
                qk_proj_ops(0, cps=[1])
                + v_proj_ops(0)[0:2]
                + qk_proj_ops(0, cps=[2])
                + v_proj_ops(0)[2:4]
                + qk_proj_ops(0, cps=[3])
                + proj_ops(1)
            )
            fast = []
            for j in range(TJ):
                nk = 4 * j + 4
                steps_left = 4 * nk
                for cp in range(EC):
                    if j == TJ - 1:
                        # chunk-3 projections staggered into j=3: stream
                        # (cp,3) drains the projections needed by (cp+1,3);
                        # out-proj(1)/(2) slack-shifted behind their ctx
                        if cp == 0:
                            slow += v_proj_ops(3) + qk_proj_ops(3, cps=[1])
                        elif cp == 1:
                            slow += qk_proj_ops(3, cps=[2]) + outproj_ops(1)
                        elif cp == 2:
                            slow += qk_proj_ops(3, cps=[3]) + outproj_ops(2)[0:2]
                        elif cp == 3:
                            slow += outproj_ops(2, scalar_copy=True)[2:4]
                        steps_left = nk
                    fper = (len(fast) + max(1, nk // 2) - 1) // max(1, nk // 2)
                    qt = get_qt(j)
                    pts = []
                    for i in range(nk):
                        r = i - 4 * j  # >=0 on the diagonal block
                        lo_q = 128 * r if r > 0 else 0
                        pt = ppool.tile([128, 2, 512], BF, tag="P")
                        sh = psS.tile([128, 2, 512], F32, tag="psS")
                        for h in range(2):
                            lo = 64 * h
                            nc.tensor.matmul(
                                sh[:, h, lo_q:512],
                                lhsT=KT_sb[lo : lo + 64, cp, i * 128 : (i + 1) * 128],
                                rhs=qt[lo : lo + 64, cp, lo_q:512],
                                start=True,
                                stop=True,
                            )
                        nc.scalar.activation(
                            out=pt[:, :, lo_q:512],
                            in_=sh[:, :, lo_q:512],
                            func=mybir.ActivationFunctionType.Exp,
                            scale=0.125,
                        )
                        if r >= 0:  # diagonal 128x128: keep f >= p, else 0
                            for h in range(2):
                                nc.gpsimd.affine_select(
                                    out=pt[:, h, 128 * r : 128 * r + 128],
                                    in_=pt[:, h, 128 * r : 128 * r + 128],
                                    compare_op=mybir.AluOpType.is_ge,
                                    fill=0.0,
                                    base=0,
                                    pattern=[[1, 128]],
                                    channel_multiplier=-1,
                                )
                        pts.append(pt)
                        for _ in range(fper):
                            if fast:
                                fast.pop(0)()
                        spr = (len(slow) + steps_left - 1) // steps_left
                        for _ in range(spr):
                            if slow:
                                slow.pop(0)()
                        steps_left -= 1
                    while fast:
                        fast.pop(0)()
                    fast = ctx_ops(cp, j, pts)
                # end of q-chunk j: stage next work into the queues
                if j == 0:
                    slow += qk_proj_ops(2)
                elif j == 1:
                    slow += v_proj_ops(2) + outproj_ops(0) + qk_proj_ops(3, cps=[0])
                if j == TJ - 1:
                    # tail: out-proj lags ctx by one quarter so the DMA
                    # transpose latency is hidden; copies on idle ScalarE
                    op4 = outproj_ops(3, scalar_copy=True)
                    mix = fast[0:3] + fast[3:6] + [op4[0]]
                    mix += fast[6:9] + [op4[1]]
                    mix += fast[9:12] + [op4[2], op4[3]]
                    fast = mix
            while fast:
                fast.pop(0)()
            while slow:
                slow.pop(0)()
    nc.compile()
    return nc


def _get_nc():
    if "nc" not in _CACHE:
        _CACHE["nc"] = _build()
    return _CACHE["nc"]


def _ensure_ntff_hook():
    """Install the axon NTFF profiling hook if the image's antenv lacks it."""
    import sys
    import types

    try:
        import antenv.axon_hooks  # noqa: F401

        return
    except ImportError:
        pass
    try:
        import antenv

        mod = types.ModuleType("antenv.axon_hooks")
        holder = {"hook": None}
        mod.set_axon_ntff_profile_hook = lambda h: holder.__setitem__("hook", h)
        mod.get_axon_ntff_profile_hook = lambda: holder["hook"]
        sys.modules["antenv.axon_hooks"] = mod
        antenv.axon_hooks = mod
        from trn_agent_boot.trn_boot import _ntff_profile_via_ctypes

        so = "/opt/axon/libaxon_pjrt.so"
        if os.path.exists(so):
            mod.set_axon_ntff_profile_hook(_ntff_profile_via_ctypes(so))
    except Exception:
        pass


def kernel(inputs, Wq, Wk, Wv, Wo, bo):
    inputs = np.asarray(inputs, dtype=np.float32)
    Wq = np.asarray(Wq, dtype=np.float32)
    Wk = np.asarray(Wk, dtype=np.float32)
    Wv = np.asarray(Wv, dtype=np.float32)
    Wo = np.asarray(Wo, dtype=np.float32)
    bo = np.asarray(bo, dtype=np.float32)

    nc = _get_nc()
    wqs = [np.ascontiguousarray(Wq[:, g * E : (g + 1) * E]).astype(BF16) for g in range(2)]
    wks = [np.ascontiguousarray(Wk[:, g * E : (g + 1) * E]).astype(BF16) for g in range(2)]
    wvs = [np.ascontiguousarray(Wv[:, g * E : (g + 1) * E]).astype(BF16) for g in range(2)]
    wos = [np.ascontiguousarray(Wo[g * E : (g + 1) * E, :]).astype(BF16) for g in range(2)]
    xTs = [np.ascontiguousarray(inputs[b].T).astype(BF16) for b in range(B)]

    in_maps = []
    for c in range(8):
        b, g = divmod(c, 2)
        in_maps.append(
            {
                "xT": xTs[b],
                "wq": wqs[g],
                "wk": wks[g],
                "wv": wvs[g],
                "wo": wos[g],
            }
        )

    trace = os.environ.get("KERNEL_TRACE", "0") == "1"
    if trace:
        _ensure_ntff_hook()
    tcores = None
    if os.environ.get("KERNEL_TRACE_ALL", "0") == "1":
        tcores = list(range(8))
    res = run_bass_kernel_spmd(
        nc, in_maps, core_ids=list(range(8)), trace=trace, trace_cores=tcores
    )
    LAST["exec_ns"] = res.exec_time_ns
    LAST["trace"] = res.instructions_and_trace
    LAST["profile_json"] = res.profile_json

    out = np.empty((B, T, D), np.float32)
    for b in range(B):
        out[b] = res.results[2 * b]["out"] + res.results[2 * b + 1]["out"] + bo[None, :]
    return out


# revision 24
# speedup vs baseline: 1.1265x; 1.0612x over previous
"""Causal MHA (B=4, T=2048, D=1024, H=16) on 8 trn2 cores.

Sharding: core c = (batch b = c//2, head-group g = c%2). Each core computes
QKV projections for its 8 heads, causal attention, and the row-parallel
out-proj partial product. Host sums the two partials per batch + bias.

Schedule: q-chunk j (512 queries) OUTER, head-pair cp INNER ("j-outer").
Per j: S^T tiles stream through PE->ScalarE(exp); projections for chunk
j+1 and out-proj for chunk j-1 drain as queued PE work between S steps,
so the tensor engine never starves and HAM stays at full clock.
Diagonal S tiles are narrowed to the causal range (less PE/ACT/mask work).

On-device layout (per core):
  xT   [1024, 2048]  X^T (d on partitions)           bf16
  QT_j [512, 512]    Q^T for current q-chunk         bf16 (2 bufs)
  KT   [512, 2048]   K^T (e=head*64+d rows)          bf16
  V_pad [2048, 520]  V natural + ones col per head   bf16
  scores S^T tiles [128 k, 2x512 q] (2 heads/psum), exp on ScalarE,
  ctx = P^T-stationary matmul -> [128 q, 65] (col 64 = softmax denom),
  normalize per-partition, DMA-transpose -> ctx^T, out-proj partial.
"""

import os

import numpy as np
import ml_dtypes

import concourse.bass as bass
import concourse.bacc as bacc
import concourse.tile as tile
from concourse import mybir
from concourse.bass_utils import run_bass_kernel_spmd
from concourse.masks import make_identity

BF16 = ml_dtypes.bfloat16

B, T, D = 4, 2048, 1024
H, HD = 16, 64
E = 512          # per-core projection width (8 heads * 64)
DC = D // 128    # 8 contraction chunks
EC = E // 128    # 4 e chunks (head pairs)
TJ = T // 512    # 4 q-chunks of 512
TQ = T // 128    # 16 t-chunks of 128

F32 = mybir.dt.float32
BF = mybir.dt.bfloat16

LAST = {}
_CACHE = {}


def _build():
    nc = bacc.Bacc("TRN2")
    # host pre-arranges everything into [partition, ...] contiguous layouts
    # so every input DMA is descriptor-minimal (128 x big-contiguous)
    xT = nc.dram_tensor("xT", [TJ, 128, DC * 512], BF, kind="ExternalInput")
    wq = nc.dram_tensor("wq", [128, DC * E], BF, kind="ExternalInput")
    wk = nc.dram_tensor("wk", [128, DC * E], BF, kind="ExternalInput")
    wv = nc.dram_tensor("wv", [128, DC * E], BF, kind="ExternalInput")
    wo = nc.dram_tensor("wo", [128, EC * D], BF, kind="ExternalInput")
    outp = nc.dram_tensor("out", [T, D], BF, kind="ExternalOutput")

    with tile.TileContext(nc) as tc:
        with (
            tc.tile_pool(name="const", bufs=1) as const,
            tc.tile_pool(name="acts", bufs=1) as acts,
            tc.tile_pool(name="ppool", bufs=26) as ppool,
            tc.tile_pool(name="small", bufs=6) as small,
            tc.tile_pool(name="stage", bufs=6) as stage,
            tc.tile_pool(name="obuf", bufs=4) as obufp,
            tc.tile_pool(name="psS", bufs=2, space="PSUM") as psS,
            tc.tile_pool(name="psP", bufs=2, space="PSUM") as psP,
            tc.tile_pool(name="psC", bufs=2, space="PSUM") as psC,
        ):
            # ---------- constants; DMA order = first-need order:
            # wq/wk + xT chunk0 gate the j=0 streams, wv next (ctx j=0),
            # remaining xT chunks gate proj of chunk j+1, wo only at outproj
            wq_sb = const.tile([128, DC, E], BF, tag="wq")
            wk_sb = const.tile([128, DC, E], BF, tag="wk")
            wv_sb = const.tile([128, DC, E], BF, tag="wv")
            wo_sb = const.tile([128, EC, D], BF, tag="wo")
            ident = const.tile([128, 128], BF, tag="ident")
            xT_sb = acts.tile([128, TJ, DC, 512], BF, tag="xT")

            # weights ride the ScalarE DMA queue (idle at startup); xT rides
            # Sync in q-chunk order; all transfers fully contiguous
            nc.scalar.dma_start(out=wq_sb.rearrange("p a b -> p (a b)"), in_=wq[:, :])
            nc.scalar.dma_start(out=wk_sb.rearrange("p a b -> p (a b)"), in_=wk[:, :])
            for t5 in range(TJ):
                nc.sync.dma_start(
                    out=xT_sb[:, t5].rearrange("p a b -> p (a b)"),
                    in_=xT[t5, :, :],
                )
            nc.scalar.dma_start(out=wv_sb.rearrange("p a b -> p (a b)"), in_=wv[:, :])
            nc.scalar.dma_start(out=wo_sb.rearrange("p a b -> p (a b)"), in_=wo[:, :])
            make_identity(nc, ident)

            # trigger the exp table load immediately (overlaps input DMA)
            actwarm = small.tile([128, 1], F32, tag="actwarm")
            nc.scalar.activation(
                out=actwarm,
                in_=ident[:, 0:1],
                func=mybir.ActivationFunctionType.Exp,
            )

            # spin the PE on the identity during the input DMA so HAM
            # un-throttles before real work arrives (~5us of N=128 matmuls)
            psW = psP.tile([128, 512], F32, tag="psP")
            for _ in range(40):
                nc.tensor.matmul(
                    psW[:, 0:128], lhsT=ident, rhs=ident, start=True, stop=True
                )

            KT_sb = acts.tile([128, EC, T], BF, tag="KT")
            V_sb = acts.tile([128, TQ, 8 * 65], BF, tag="V")
            CT_sb = acts.tile([128, EC, T], BF, tag="CT")

            # ones columns (col 64 of each per-head 65-group): one strided memset
            nc.vector.memset(
                V_sb.rearrange("p t (h d) -> p t h d", d=65)[:, :, :, 64:65], 1.0
            )

            qt_tiles = {}

            def get_qt(j):
                if j not in qt_tiles:
                    qt_tiles[j] = acts.tile(
                        [128, EC, 512], BF, tag="QT", bufs=2, name=f"QT{j}"
                    )
                return qt_tiles[j]

            # ---------- op factories ----------
            def q_proj_op(cp, j):
                def op(cp=cp, j=j):
                    qt = get_qt(j)
                    ps = psP.tile([128, 512], F32, tag="psP")
                    for dc in range(DC):
                        nc.tensor.matmul(
                            ps,
                            lhsT=wq_sb[:, dc, cp * 128 : (cp + 1) * 128],
                            rhs=xT_sb[:, j, dc, :],
                            start=(dc == 0),
                            stop=(dc == DC - 1),
                        )
                    nc.vector.tensor_copy(out=qt[:, cp, :], in_=ps)

                return op

            def k_proj_op(cp, j):
                def op(cp=cp, j=j):
                    ps = psP.tile([128, 512], F32, tag="psP")
                    for dc in range(DC):
                        nc.tensor.matmul(
                            ps,
                            lhsT=wk_sb[:, dc, cp * 128 : (cp + 1) * 128],
                            rhs=xT_sb[:, j, dc, :],
                            start=(dc == 0),
                            stop=(dc == DC - 1),
                        )
                    nc.vector.tensor_copy(
                        out=KT_sb[:, cp, j * 512 : (j + 1) * 512], in_=ps
                    )

                return op

            def v_proj_op(t7):
                def op(t7=t7):
                    ps = psP.tile([128, 512], F32, tag="psP")
                    for dc in range(DC):
                        nc.tensor.matmul(
                            ps,
                            lhsT=xT_sb[:, t7 // 4, dc, (t7 % 4) * 128 : (t7 % 4) * 128 + 128],
                            rhs=wv_sb[:, dc, :],
                            start=(dc == 0),
                            stop=(dc == DC - 1),
                        )
                    nc.vector.tensor_copy(
                        out=V_sb.rearrange("p t (h d) -> p t h d", d=65)[
                            :, t7, :, 0:64
                        ],
                        in_=ps.rearrange("p (h d) -> p h d", d=64),
                    )

                return op

            def qk_proj_ops(j, cps=None):
                ops = []
                for cp in cps if cps is not None else range(EC):
                    ops.append(q_proj_op(cp, j))
                    ops.append(k_proj_op(cp, j))
                return ops

            def v_proj_ops(j):
                return [v_proj_op(t7) for t7 in range(4 * j, 4 * j + 4)]

            def proj_ops(j):
                return qk_proj_ops(j) + v_proj_ops(j)

            def outproj_ops(j, scalar_copy=False):
                ops = []
                for t7 in range(4 * j, 4 * j + 4):

                    def op(t7=t7, scalar_copy=scalar_copy):
                        ob = obufp.tile([128, 1024], BF, tag="obuf")
                        for oc in range(2):
                            ps = psP.tile([128, 512], F32, tag="psP")
                            for ec in range(EC):
                                nc.tensor.matmul(
                                    ps,
                                    lhsT=CT_sb[:, ec, t7 * 128 : (t7 + 1) * 128],
                                    rhs=wo_sb[:, ec, oc * 512 : (oc + 1) * 512],
                                    start=(ec == 0),
                                    stop=(ec == EC - 1),
                                )
                            if scalar_copy:
                                nc.scalar.copy(
                                    out=ob[:, oc * 512 : (oc + 1) * 512], in_=ps
                                )
                            else:
                                nc.vector.tensor_copy(
                                    out=ob[:, oc * 512 : (oc + 1) * 512], in_=ps
                                )
                        nc.sync.dma_start(
                            out=outp[t7 * 128 : (t7 + 1) * 128, :], in_=ob
                        )

                    ops.append(op)
                return ops

            def ctx_ops(cp, j, pts):
                ops = []
                for qr in range(4):
                    qc = 4 * j + qr
                    cn = stage.tile([128, 128], BF, tag="ctxn")
                    for h in range(2):

                        def mm_group(cp=cp, h=h, qr=qr, qc=qc, j=j, pts=pts, cn=cn):
                            habs = 2 * cp + h
                            cps = psC.tile([128, 65], F32, tag="psC")
                            for i in range(qc + 1):
                                nc.tensor.matmul(
                                    cps,
                                    lhsT=pts[i][:, h, qr * 128 : (qr + 1) * 128],
                                    rhs=V_sb[:, i, habs * 65 : habs * 65 + 65],
                                    start=(i == 0),
                                    stop=(i == qc),
                                )
                            rc = small.tile([128, 1], F32, tag="recip")
                            nc.vector.reciprocal(rc, cps[:, 64:65])
                            nc.vector.tensor_scalar_mul(
                                out=cn[:, 64 * h : 64 * h + 64],
                                in0=cps[:, 0:64],
                                scalar1=rc,
                            )

                        ops.append(mm_group)

                    def finish(cp=cp, qc=qc, cn=cn):
                        nc.sync.dma_start_transpose(
                            out=CT_sb[:, cp, qc * 128 : (qc + 1) * 128], in_=cn
                        )

                    ops.append(finish)
                return ops

            # ---------- pipelined emission ----------
            # only the first stream's projections go inline; everything else
            # drains between S steps so ScalarE starts exp'ing ASAP
            for op in qk_proj_ops(0, cps=[0]):
                op()

            slow = (
                qk_proj_ops(0, cps=[1])
                + v_proj_ops(0)[0:2]
                + qk_proj_ops(0, cps=[2])
                + v_proj_ops(0)[2:4]
                + qk_proj_ops(0, cps=[3])
                + proj_ops(1)
            )
            fast = []
            for j in range(TJ):
                nk = 4 * j + 4
                steps_left = 4 * nk
                for cp in range(EC):
                    if j == TJ - 1:
                        # chunk-3 projections staggered into j=3: stream
                        # (cp,3) drains the projections needed by (cp+1,3);
                        # out-proj(1)/(2) slack-shifted behind their ctx
                        if cp == 0:
                            slow += v_proj_ops(3) + qk_proj_ops(3, cps=[1])
                        elif cp == 1:
                            slow += qk_proj_ops(3, cps=[2]) + outproj_ops(1)
                        elif cp == 2:
                            slow += qk_proj_ops(3, cps=[3]) + outproj_ops(2)[0:2]
                        elif cp == 3:
                            slow += outproj_ops(2, scalar_copy=True)[2:4]
                        steps_left = nk
                    fper = (len(fast) + max(1, nk // 2) - 1) // max(1, nk // 2)
                    qt = get_qt(j)
                    pts = []
                    for i in range(nk):
                        r = i - 4 * j  # >=0 on the diagonal block
                        lo_q = 128 * r if r > 0 else 0
                        pt = ppool.tile([128, 2, 512], BF, tag="P")
                        sh = psS.tile([128, 2, 512], F32, tag="psS")
                        for h in range(2):
                            lo = 64 * h
                            nc.tensor.matmul(
                                sh[:, h, lo_q:512],
                                lhsT=KT_sb[lo : lo + 64, cp, i * 128 : (i + 1) * 128],
                                rhs=qt[lo : lo + 64, cp, lo_q:512],
                                start=True,
                                stop=True,
                            )
                        nc.scalar.activation(
                            out=pt[:, :, lo_q:512],
                            in_=sh[:, :, lo_q:512],
                            func=mybir.ActivationFunctionType.Exp,
                            scale=0.125,
                        )
                        if r >= 0:  # diagonal 128x128: keep f >= p, else 0
                            for h in range(2):
                                nc.gpsimd.affine_select(
                                    out=pt[:, h, 128 * r : 128 * r + 128],
                                    in_=pt[:, h, 128 * r : 128 * r + 128],
                                    compare_op=mybir.AluOpType.is_ge,
                                    fill=0.0,
                                    base=0,
                                    pattern=[[1, 128]],
                                    channel_multiplier=-1,
                                )
                        pts.append(pt)
                        for _ in range(fper):
                            if fast:
                                fast.pop(0)()
                        spr = (len(slow) + steps_left - 1) // steps_left
                        for _ in range(spr):
                            if slow:
                                slow.pop(0)()
                        steps_left -= 1
                    while fast:
                        fast.pop(0)()
                    fast = ctx_ops(cp, j, pts)
                # end of q-chunk j: stage next work into the queues
                if j == 0:
                    slow += qk_proj_ops(2)
                elif j == 1:
                    slow += v_proj_ops(2) + outproj_ops(0) + qk_proj_ops(3, cps=[0])
                if j == TJ - 1:
                    # tail: out-proj lags ctx by one quarter so the DMA
                    # transpose latency is hidden; copies on idle ScalarE
                    op4 = outproj_ops(3, scalar_copy=True)
                    mix = fast[0:3] + fast[3:6] + [op4[0]]
                    mix += fast[6:9] + [op4[1]]
                    mix += fast[9:12] + [op4[2], op4[3]]
                    fast = mix
            while fast:
                fast.pop(0)()
            while slow:
                slow.pop(0)()
    nc.compile()
    return nc


def _get_nc():
    if "nc" not in _CACHE:
        _CACHE["nc"] = _build()
    return _CACHE["nc"]


def _ensure_ntff_hook():
    """Install the axon NTFF profiling hook if the image's antenv lacks it."""
    import sys
    import types

    try:
        import antenv.axon_hooks  # noqa: F401

        return
    except ImportError:
        pass
    try:
        import antenv

        mod = types.ModuleType("antenv.axon_hooks")
        holder = {"hook": None}
        mod.set_axon_ntff_profile_hook = lambda h: holder.__setitem__("hook", h)
        mod.get_axon_ntff_profile_hook = lambda: holder["hook"]
        sys.modules["antenv.axon_hooks"] = mod
        antenv.axon_hooks = mod
        from trn_agent_boot.trn_boot import _ntff_profile_via_ctypes

        so = "/opt/axon/libaxon_pjrt.so"
        if os.path.exists(so):
            mod.set_axon_ntff_profile_hook(_ntff_profile_via_ctypes(so))
    except Exception:
        pass


def kernel(inputs, Wq, Wk, Wv, Wo, bo):
    inputs = np.asarray(inputs, dtype=np.float32)
    Wq = np.asarray(Wq, dtype=np.float32)
    Wk = np.asarray(Wk, dtype=np.float32)
    Wv = np.asarray(Wv, dtype=np.float32)
    Wo = np.asarray(Wo, dtype=np.float32)
    bo = np.asarray(bo, dtype=np.float32)

    nc = _get_nc()

    def warr(w):  # [D, E] -> [128, DC*E] partition-major contiguous
        return np.ascontiguousarray(
            w.reshape(DC, 128, E).transpose(1, 0, 2).reshape(128, DC * E)
        ).astype(BF16)

    def woarr(w):  # [E, D] -> [128, EC*D]
        return np.ascontiguousarray(
            w.reshape(EC, 128, D).transpose(1, 0, 2).reshape(128, EC * D)
        ).astype(BF16)

    def xarr(xb):  # [T, D] -> [TJ, 128, DC*512]
        a = xb.T.reshape(DC, 128, TJ, 512).transpose(2, 1, 0, 3)
        return np.ascontiguousarray(a.reshape(TJ, 128, DC * 512)).astype(BF16)

    wqs = [warr(Wq[:, g * E : (g + 1) * E]) for g in range(2)]
    wks = [warr(Wk[:, g * E : (g + 1) * E]) for g in range(2)]
    wvs = [warr(Wv[:, g * E : (g + 1) * E]) for g in range(2)]
    wos = [woarr(Wo[g * E : (g + 1) * E, :]) for g in range(2)]
    xTs = [xarr(inputs[b]) for b in range(B)]

    in_maps = []
    for c in range(8):
        b, g = divmod(c, 2)
        in_maps.append(
            {
                "xT": xTs[b],
                "wq": wqs[g],
                "wk": wks[g],
                "wv": wvs[g],
                "wo": wos[g],
            }
        )

    trace = os.environ.get("KERNEL_TRACE", "0") == "1"
    if trace:
        _ensure_ntff_hook()
    tcores = None
    if os.environ.get("KERNEL_TRACE_ALL", "0") == "1":
        tcores = list(range(8))
    res = run_bass_kernel_spmd(
        nc, in_maps, core_ids=list(range(8)), trace=trace, trace_cores=tcores
    )
    LAST["exec_ns"] = res.exec_time_ns
    LAST["trace"] = res.instructions_and_trace
    LAST["profile_json"] = res.profile_json

    out = np.empty((B, T, D), np.float32)
    for b in range(B):
        out[b] = (
            res.results[2 * b]["out"].astype(np.float32)
            + res.results[2 * b + 1]["out"].astype(np.float32)
            + bo[None, :]
        )
    return out


# revision 39
# speedup vs baseline: 1.1452x; 1.0166x over previous
"""Causal MHA (B=4, T=2048, D=1024, H=16) on 8 trn2 cores.

Sharding: core c = (batch b = c//2, head-group g = c%2). Each core computes
QKV projections for its 8 heads, causal attention, and the row-parallel
out-proj partial product. Host sums the two partials per batch + bias.

Schedule: q-chunk j (512 queries) OUTER, head-pair cp INNER ("j-outer").
Per j: S^T tiles stream through PE->ScalarE(exp); projections for chunk
j+1 and out-proj for chunk j-1 drain as queued PE work between S steps,
so the tensor engine never starves and HAM stays at full clock.
Diagonal S tiles are narrowed to the causal range (less PE/ACT/mask work).

On-device layout (per core):
  xT   [1024, 2048]  X^T (d on partitions)           bf16
  QT_j [512, 512]    Q^T for current q-chunk         bf16 (2 bufs)
  KT   [512, 2048]   K^T (e=head*64+d rows)          bf16
  V_pad [2048, 520]  V natural + ones col per head   bf16
  scores S^T tiles [128 k, 2x512 q] (2 heads/psum), exp on ScalarE,
  ctx = P^T-stationary matmul -> [128 q, 65] (col 64 = softmax denom),
  normalize per-partition, DMA-transpose -> ctx^T, out-proj partial.
"""

import os

import numpy as np
import ml_dtypes

import concourse.bass as bass
import concourse.bacc as bacc
import concourse.tile as tile
from concourse import mybir
from concourse.bass_utils import run_bass_kernel_spmd
from concourse.masks import make_identity

BF16 = ml_dtypes.bfloat16

B, T, D = 4, 2048, 1024
H, HD = 16, 64
E = 512          # per-core projection width (8 heads * 64)
DC = D // 128    # 8 contraction chunks
EC = E // 128    # 4 e chunks (head pairs)
TJ = T // 512    # 4 q-chunks of 512
TQ = T // 128    # 16 t-chunks of 128

F32 = mybir.dt.float32
BF = mybir.dt.bfloat16

LAST = {}
_CACHE = {}


def _build():
    nc = bacc.Bacc("TRN2")
    # host pre-arranges everything into [partition, ...] contiguous layouts
    # so every input DMA is descriptor-minimal (128 x big-contiguous)
    xT = nc.dram_tensor("xT", [TJ, 128, DC * 512], BF, kind="ExternalInput")
    wq = nc.dram_tensor("wq", [128, DC * E], BF, kind="ExternalInput")
    wk = nc.dram_tensor("wk", [128, DC * E], BF, kind="ExternalInput")
    wv = nc.dram_tensor("wv", [128, DC * E], BF, kind="ExternalInput")
    wo = nc.dram_tensor("wo", [128, EC * D], BF, kind="ExternalInput")
    outp = nc.dram_tensor("out", [T, D], BF, kind="ExternalOutput")

    with tile.TileContext(nc) as tc:
        with (
            tc.tile_pool(name="const", bufs=1) as const,
            tc.tile_pool(name="acts", bufs=1) as acts,
            tc.tile_pool(name="ppool", bufs=26) as ppool,
            tc.tile_pool(name="small", bufs=6) as small,
            tc.tile_pool(name="stage", bufs=6) as stage,
            tc.tile_pool(name="obuf", bufs=4) as obufp,
            tc.tile_pool(name="psS", bufs=2, space="PSUM") as psS,
            tc.tile_pool(name="psP", bufs=2, space="PSUM") as psP,
            tc.tile_pool(name="psC", bufs=2, space="PSUM") as psC,
        ):
            # ---------- constants; DMA order = first-need order:
            # wq/wk + xT chunk0 gate the j=0 streams, wv next (ctx j=0),
            # remaining xT chunks gate proj of chunk j+1, wo only at outproj
            wq_sb = const.tile([128, DC, E], BF, tag="wq")
            wk_sb = const.tile([128, DC, E], BF, tag="wk")
            wv_sb = const.tile([128, DC, E], BF, tag="wv")
            wo_sb = const.tile([128, EC, D], BF, tag="wo")
            ident = const.tile([128, 128], BF, tag="ident")
            xT_sb = acts.tile([128, TJ, DC, 512], BF, tag="xT")

            # weights ride the ScalarE DMA queue (idle at startup); xT rides
            # Sync in q-chunk order; all transfers fully contiguous
            # only the DMAs gating the first stream go up front; the rest are
            # emitted at later loop boundaries so the first 3MB get full HBM
            # bandwidth (transfers on different queues compete for BW)
            def dma_xT(t5):
                nc.sync.dma_start(
                    out=xT_sb[:, t5].rearrange("p a b -> p (a b)"),
                    in_=xT[t5, :, :],
                )

            nc.scalar.dma_start(out=wq_sb.rearrange("p a b -> p (a b)"), in_=wq[:, :])
            nc.scalar.dma_start(out=wk_sb.rearrange("p a b -> p (a b)"), in_=wk[:, :])
            dma_xT(0)
            make_identity(nc, ident)

            # trigger the exp table load immediately (overlaps input DMA)
            actwarm = small.tile([128, 1], F32, tag="actwarm")
            nc.scalar.activation(
                out=actwarm,
                in_=ident[:, 0:1],
                func=mybir.ActivationFunctionType.Exp,
            )

            # spin the PE on the identity during the input DMA so HAM
            # un-throttles before real work arrives (~5us of N=128 matmuls)
            psW = psP.tile([128, 512], F32, tag="psP")
            for _ in range(90):
                nc.tensor.matmul(
                    psW[:, 0:128], lhsT=ident, rhs=ident, start=True, stop=True
                )

            KT_sb = acts.tile([128, EC, T], BF, tag="KT")
            V_sb = acts.tile([128, TQ, 8 * 65], BF, tag="V")
            CT_sb = acts.tile([128, EC, T], BF, tag="CT")

            # ones columns (col 64 of each per-head 65-group): one strided memset
            nc.vector.memset(
                V_sb.rearrange("p t (h d) -> p t h d", d=65)[:, :, :, 64:65], 1.0
            )

            qt_tiles = {}

            def get_qt(j):
                if j not in qt_tiles:
                    qt_tiles[j] = acts.tile(
                        [128, EC, 512], BF, tag="QT", bufs=2, name=f"QT{j}"
                    )
                return qt_tiles[j]

            # ---------- op factories ----------
            # every queued op is <=4 matmuls (~900ns of PE) so an S-step's
            # drains never push the next S-MM far enough back to bubble ACT
            def _chain2(mk_mm, finish):
                cell = {}

                def op_a():
                    ps = cell["ps"] = psP.tile([128, 512], F32, tag="psP", name="ps")
                    for dc in range(4):
                        mk_mm(ps, dc)

                def op_b():
                    ps = cell["ps"]
                    for dc in range(4, DC):
                        mk_mm(ps, dc)
                    finish(ps)

                return [op_a, op_b]

            def q_proj_op(cp, j):
                def mk(ps, dc, cp=cp, j=j):
                    nc.tensor.matmul(
                        ps,
                        lhsT=wq_sb[:, dc, cp * 128 : (cp + 1) * 128],
                        rhs=xT_sb[:, j, dc, :],
                        start=(dc == 0),
                        stop=(dc == DC - 1),
                    )

                def fin(ps, cp=cp, j=j):
                    qt = get_qt(j)
                    nc.vector.tensor_copy(out=qt[:, cp, :], in_=ps)

                return _chain2(mk, fin)

            def k_proj_op(cp, j):
                def mk(ps, dc, cp=cp, j=j):
                    nc.tensor.matmul(
                        ps,
                        lhsT=wk_sb[:, dc, cp * 128 : (cp + 1) * 128],
                        rhs=xT_sb[:, j, dc, :],
                        start=(dc == 0),
                        stop=(dc == DC - 1),
                    )

                def fin(ps, cp=cp, j=j):
                    nc.vector.tensor_copy(
                        out=KT_sb[:, cp, j * 512 : (j + 1) * 512], in_=ps
                    )

                return _chain2(mk, fin)

            def v_proj_op(t7):
                def mk(ps, dc, t7=t7):
                    nc.tensor.matmul(
                        ps,
                        lhsT=xT_sb[:, t7 // 4, dc, (t7 % 4) * 128 : (t7 % 4) * 128 + 128],
                        rhs=wv_sb[:, dc, :],
                        start=(dc == 0),
                        stop=(dc == DC - 1),
                    )

                def fin(ps, t7=t7):
                    nc.vector.tensor_copy(
                        out=V_sb.rearrange("p t (h d) -> p t h d", d=65)[
                            :, t7, :, 0:64
                        ],
                        in_=ps.rearrange("p (h d) -> p h d", d=64),
                    )

                return _chain2(mk, fin)

            def qk_proj_ops(j, cps=None):
                ops = []
                for cp in cps if cps is not None else range(EC):
                    ops += [(("qk", j, cp), o) for o in q_proj_op(cp, j)]
                    ops += [(("qk", j, cp), o) for o in k_proj_op(cp, j)]
                return ops

            def v_proj_ops(j):
                ops = []
                for t7 in range(4 * j, 4 * j + 4):
                    ops += [(("v", j), o) for o in v_proj_op(t7)]
                return ops

            def proj_ops(j):
                return qk_proj_ops(j) + v_proj_ops(j)

            def outproj_ops(j, scalar_copy=False):
                ops = []
                for t7 in range(4 * j, 4 * j + 4):
                    cell = {}

                    def op_oc(t7, oc, scalar_copy, cell):
                        def op():
                            if oc == 0:
                                cell["ob"] = obufp.tile(
                                    [128, 1024], BF, tag="obuf", name="ob"
                                )
                            ob = cell["ob"]
                            ps = psP.tile([128, 512], F32, tag="psP")
                            for ec in range(EC):
                                nc.tensor.matmul(
                                    ps,
                                    lhsT=CT_sb[:, ec, t7 * 128 : (t7 + 1) * 128],
                                    rhs=wo_sb[:, ec, oc * 512 : (oc + 1) * 512],
                                    start=(ec == 0),
                                    stop=(ec == EC - 1),
                                )
                            if scalar_copy:
                                nc.scalar.copy(
                                    out=ob[:, oc * 512 : (oc + 1) * 512], in_=ps
                                )
                            else:
                                nc.vector.tensor_copy(
                                    out=ob[:, oc * 512 : (oc + 1) * 512], in_=ps
                                )
                            if oc == 1:
                                nc.sync.dma_start(
                                    out=outp[t7 * 128 : (t7 + 1) * 128, :], in_=ob
                                )

                        return op

                    ops.append((("op", j), op_oc(t7, 0, scalar_copy, cell)))
                    ops.append((("op", j), op_oc(t7, 1, scalar_copy, cell)))
                return ops

            def ctx_ops(cp, j, pts):
                ops = []
                for qr in range(4):
                    qc = 4 * j + qr
                    cn = stage.tile([128, 128], BF, tag="ctxn")
                    for h in range(2):

                        def mm_group(cp=cp, h=h, qr=qr, qc=qc, j=j, pts=pts, cn=cn):
                            habs = 2 * cp + h
                            cps = psC.tile([128, 65], F32, tag="psC")
                            for i in range(qc + 1):
                                nc.tensor.matmul(
                                    cps,
                                    lhsT=pts[i][:, h, qr * 128 : (qr + 1) * 128],
                                    rhs=V_sb[:, i, habs * 65 : habs * 65 + 65],
                                    start=(i == 0),
                                    stop=(i == qc),
                                )
                            rc = small.tile([128, 1], F32, tag="recip")
                            nc.vector.reciprocal(rc, cps[:, 64:65])
                            nc.vector.tensor_scalar_mul(
                                out=cn[:, 64 * h : 64 * h + 64],
                                in0=cps[:, 0:64],
                                scalar1=rc,
                            )

                        ops.append(mm_group)

                    def finish(cp=cp, qc=qc, cn=cn):
                        nc.sync.dma_start_transpose(
                            out=CT_sb[:, cp, qc * 128 : (qc + 1) * 128], in_=cn
                        )

                    ops.append(finish)
                return ops

            # ---------- pipelined emission ----------
            # only the first stream's projections go inline; everything else
            # drains between S steps so ScalarE starts exp'ing ASAP
            for _, op in qk_proj_ops(0, cps=[0]):
                op()
            # remaining input DMAs, now that the critical ones have full BW
            nc.scalar.dma_start(out=wv_sb.rearrange("p a b -> p (a b)"), in_=wv[:, :])
            dma_xT(1)

            slow = (
                qk_proj_ops(0, cps=[1])
                + v_proj_ops(0)[0:4]
                + qk_proj_ops(0, cps=[2])
                + v_proj_ops(0)[4:8]
                + qk_proj_ops(0, cps=[3])
                + proj_ops(1)
            )
            fast = []
            for j in range(TJ):
                if j == 1:
                    dma_xT(2)
                    nc.scalar.dma_start(
                        out=wo_sb.rearrange("p a b -> p (a b)"), in_=wo[:, :]
                    )
                elif j == 2:
                    dma_xT(3)
                nk = 4 * j + 4
                steps_left = 4 * nk
                for cp in range(EC):
                    if j == TJ - 1:
                        # chunk-3 projections staggered into j=3: stream
                        # (cp,3) drains the projections needed by (cp+1,3);
                        # out-proj(1)/(2) slack-shifted behind their ctx
                        if cp == 0:
                            slow += v_proj_ops(3) + qk_proj_ops(3, cps=[1])
                        elif cp == 1:
                            slow += qk_proj_ops(3, cps=[2]) + outproj_ops(1)
                        elif cp == 2:
                            slow += qk_proj_ops(3, cps=[3]) + outproj_ops(2)[0:4]
                        elif cp == 3:
                            slow += outproj_ops(2, scalar_copy=True)[4:8]
                        steps_left = nk
                    fper = (len(fast) + max(1, nk // 2) - 1) // max(1, nk // 2)
                    # this stream's own Q/K projections must be emitted first
                    while any(
                        t[0] == "qk" and t[1] == j and t[2] == cp for t, _ in slow
                    ):
                        slow.pop(0)[1]()
                    qt = get_qt(j)
                    pts = []
                    for i in range(nk):
                        r = i - 4 * j  # >=0 on the diagonal block
                        lo_q = 128 * r if r > 0 else 0
                        pt = ppool.tile([128, 2, 512], BF, tag="P")
                        sh = psS.tile([128, 2, 512], F32, tag="psS")
                        for h in range(2):
                            lo = 64 * h
                            nc.tensor.matmul(
                                sh[:, h, lo_q:512],
                                lhsT=KT_sb[lo : lo + 64, cp, i * 128 : (i + 1) * 128],
                                rhs=qt[lo : lo + 64, cp, lo_q:512],
                                start=True,
                                stop=True,
                            )
                        nc.scalar.activation(
                            out=pt[:, :, lo_q:512],
                            in_=sh[:, :, lo_q:512],
                            func=mybir.ActivationFunctionType.Exp,
                            scale=0.125,
                        )
                        if r >= 0:  # diagonal 128x128: keep f >= p, else 0
                            for h in range(2):
                                nc.gpsimd.affine_select(
                                    out=pt[:, h, 128 * r : 128 * r + 128],
                                    in_=pt[:, h, 128 * r : 128 * r + 128],
                                    compare_op=mybir.AluOpType.is_ge,
                                    fill=0.0,
                                    base=0,
                                    pattern=[[1, 128]],
                                    channel_multiplier=-1,
                                )
                        pts.append(pt)
                        for _ in range(fper):
                            if fast:
                                fast.pop(0)()
                        spr = (len(slow) + steps_left - 1) // steps_left
                        for _ in range(spr):
                            if slow:
                                slow.pop(0)[1]()
                        steps_left -= 1
                    while fast:
                        fast.pop(0)()
                    # ctx(cp,j) reads V chunks <= j: force-emit any remaining
                    # V producers before the ctx ops can pop (FIFO safety)
                    while any(t[0] == "v" and t[1] <= j for t, _ in slow):
                        slow.pop(0)[1]()
                    fast = ctx_ops(cp, j, pts)
                # end of q-chunk j: stage next work into the queues
                if j == 0:
                    slow += qk_proj_ops(2)
                elif j == 1:
                    slow += v_proj_ops(2) + outproj_ops(0) + qk_proj_ops(3, cps=[0])
                if j == TJ - 1:
                    # tail: out-proj lags ctx by one quarter so the DMA
                    # transpose latency is hidden; copies on idle ScalarE
                    op4 = [o for _, o in outproj_ops(3, scalar_copy=True)]
                    mix = fast[0:3] + fast[3:6] + op4[0:2]
                    mix += fast[6:9] + op4[2:4]
                    mix += fast[9:12] + op4[4:8]
                    fast = mix
            while fast:
                fast.pop(0)()
            while slow:
                slow.pop(0)[1]()
    nc.compile()
    return nc


def _get_nc():
    if "nc" not in _CACHE:
        _CACHE["nc"] = _build()
    return _CACHE["nc"]


def _ensure_ntff_hook():
    """Install the axon NTFF profiling hook if the image's antenv lacks it."""
    import sys
    import types

    try:
        import antenv.axon_hooks  # noqa: F401

        return
    except ImportError:
        pass
    try:
        import antenv

        mod = types.ModuleType("antenv.axon_hooks")
        holder = {"hook": None}
        mod.set_axon_ntff_profile_hook = lambda h: holder.__setitem__("hook", h)
        mod.get_axon_ntff_profile_hook = lambda: holder["hook"]
        sys.modules["antenv.axon_hooks"] = mod
        antenv.axon_hooks = mod
        from trn_agent_boot.trn_boot import _ntff_profile_via_ctypes

        so = "/opt/axon/libaxon_pjrt.so"
        if os.path.exists(so):
            mod.set_axon_ntff_profile_hook(_ntff_profile_via_ctypes(so))
    except Exception:
        pass


def kernel(inputs, Wq, Wk, Wv, Wo, bo):
    inputs = np.asarray(inputs, dtype=np.float32)
    Wq = np.asarray(Wq, dtype=np.float32)
    Wk = np.asarray(Wk, dtype=np.float32)
    Wv = np.asarray(Wv, dtype=np.float32)
    Wo = np.asarray(Wo, dtype=np.float32)
    bo = np.asarray(bo, dtype=np.float32)

    nc = _get_nc()

    def warr(w):  # [D, E] -> [128, DC*E] partition-major contiguous
        return np.ascontiguousarray(
            w.reshape(DC, 128, E).transpose(1, 0, 2).reshape(128, DC * E)
        ).astype(BF16)

    def woarr(w):  # [E, D] -> [128, EC*D]
        return np.ascontiguousarray(
            w.reshape(EC, 128, D).transpose(1, 0, 2).reshape(128, EC * D)
        ).astype(BF16)

    def xarr(xb):  # [T, D] -> [TJ, 128, DC*512]
        a = xb.T.reshape(DC, 128, TJ, 512).transpose(2, 1, 0, 3)
        return np.ascontiguousarray(a.reshape(TJ, 128, DC * 512)).astype(BF16)

    wqs = [warr(Wq[:, g * E : (g + 1) * E]) for g in range(2)]
    wks = [warr(Wk[:, g * E : (g + 1) * E]) for g in range(2)]
    wvs = [warr(Wv[:, g * E : (g + 1) * E]) for g in range(2)]
    wos = [woarr(Wo[g * E : (g + 1) * E, :]) for g in range(2)]
    xTs = [xarr(inputs[b]) for b in range(B)]

    in_maps = []
    for c in range(8):
        b, g = divmod(c, 2)
        in_maps.append(
            {
                "xT": xTs[b],
                "wq": wqs[g],
                "wk": wks[g],
                "wv": wvs[g],
                "wo": wos[g],
            }
        )

    trace = os.environ.get("KERNEL_TRACE", "0") == "1"
    if trace:
        _ensure_ntff_hook()
    tcores = None
    if os.environ.get("KERNEL_TRACE_ALL", "0") == "1":
        tcores = list(range(8))
    res = run_bass_kernel_spmd(
        nc, in_maps, core_ids=list(range(8)), trace=trace, trace_cores=tcores
    )
    LAST["exec_ns"] = res.exec_time_ns
    LAST["trace"] = res.instructions_and_trace
    LAST["profile_json"] = res.profile_json

    out = np.empty((B, T, D), np.float32)
    for b in range(B):
        out[b] = (
            res.results[2 * b]["out"].astype(np.float32)
            + res.results[2 * b + 1]["out"].astype(np.float32)
            + bo[None, :]
        )
    return out
